# revision 10
# baseline (speedup 1.0000x reference)
"""Tensor-parallel GQA attention block (AtlasAttentionWrapper) on 8 TRN2 cores.

Sharding: TP over heads. Core m owns query heads [4m..4m+3] (Wq rows
m*512:(m+1)*512) and KV head m (Wk/Wv rows m*128:(m+1)*128, past_k/past_v
head m).

o_proj strategy (v2): instead of ReduceScattering the 8MB o_proj partials
(CC-serialized ~128us, ~60us exposed tail in v1), the cores AllToAll the
*attention outputs* (2 x 512KB total) and each core computes the o_proj for
its own 128 output rows with the FULL 4096-dim contraction, streaming the
full Wo (32MB, replicated input) through SBUF. Comm drops 8x and is fully
hidden; core m returns final rows [128m:128(m+1)) (host concatenates).

Host pre-packs every input so each tensor is one (or a few) large contiguous
DMAs ([128, ...] partition-major): v1 spent ~70us serializing ~211 small DMA
descriptor issues at startup.

Phase 1 is c-outer: per x-chunk c, the K/V/Q(head0) projections each issue
their 2 accumulating matmuls (6 PSUM banks), so the PE starts ~3us in and
paces with the x DMA stream instead of waiting for full tensors.

All matmuls bf16 with f32 PSUM accumulation. Scores are built transposed
(S^T[kv, q] = K Q^T) so the exp() lands in the [kv, q] layout the PV matmul
consumes; the softmax denominator comes from a ones-row matmul; the 1/sum
normalization is fused into the PSUM->SBUF copy of attn^T. No
max-subtraction: score scale is ~N(0, 1.7), exp() is safe in f32/bf16.
"""

import sys

if "/opt/trn_rl_repo" not in sys.path:
    sys.path.insert(0, "/opt/trn_rl_repo")

from contextlib import ExitStack

import ml_dtypes
import numpy as np

import concourse.bass as bass
import concourse.tile as tile
from concourse import bacc, mybir
from concourse.bass import ds, ts
from concourse.bass_utils import run_bass_kernel_spmd
from concourse.masks import make_identity

NCORES = 8
B, SQ, H = 1, 1024, 4096
NH, NKV, D = 32, 8, 128
SP = 1024
KV = SP + SQ  # 2048
HPC = NH // NCORES  # 4 query heads per core
DQ = HPC * D  # 512
OWN = SQ // NCORES  # 128 output rows owned per core
ROPE_THETA = 10000.0
INV_SQRT_D = 1.0 / float(np.sqrt(D))

BF16 = mybir.dt.bfloat16
F32 = mybir.dt.float32
HCH = H // 128  # 32 contraction chunks for the projections
KVCH = KV // 128  # 16 kv chunks
EXP = mybir.ActivationFunctionType.Exp

LAST_RESULT = None
_NC_CACHE = {}

# o_proj contraction-chunk consumption order: global head chunk 4*j+h for
# (core j, local head h). A2A#1 carries heads {0,1}, A2A#2 heads {2,3};
# consume #1's 16 chunks first so A2A#2 hides under them.
CONS = [(j, h) for h in (0, 1) for j in range(NCORES)] + [
    (j, h) for h in (2, 3) for j in range(NCORES)
]
WO_GROUPS = 8  # 4 chunks per group, one wo pool buf each


def _rope_write(nc, tmp_pool, dst, src, cos_sb, sin_sb, pos, width):
    """dst[d, s] = rope(src)[d, s]; cos/sin tables indexed at pos (table-rel).

    src: AP [128, width] (PSUM f32), dst: SBUF bf16 AP.
    rope: out[d<64] = x[d]*cos[d] - x[d+64]*sin[d]
          out[d>=64] = x[d]*cos[d] + x[d-64]*sin[d]
    """
    cs = cos_sb[:, ds(pos, width)]
    sn = sin_sb[:, ds(pos, width)]
    t = tmp_pool.tile([128, width], F32, tag="rope_t")
    u = tmp_pool.tile([128, width], F32, tag="rope_u")
    nc.vector.tensor_mul(t[0:64, :], src[64:128, :], sn[0:64, :])
    nc.vector.tensor_mul(t[64:128, :], src[0:64, :], sn[64:128, :])
    nc.vector.tensor_mul(u[:, :], src[:, :], cs)
    nc.vector.tensor_sub(dst[0:64, :], u[0:64, :], t[0:64, :])
    nc.vector.tensor_add(dst[64:128, :], u[64:128, :], t[64:128, :])


def _build_nc():
    nc = bacc.Bacc(None, target_bir_lowering=False, debug=False)

    # Host-packed DRAM inputs (partition-major, one contiguous DMA each).
    xT = nc.declare_dram_parameter("xT", [128, HCH, SQ], BF16, False)
    wkT = nc.declare_dram_parameter("wkT", [128, HCH, D], BF16, False)
    wvT = nc.declare_dram_parameter("wvT", [128, HCH, D], BF16, False)
    wqT = nc.declare_dram_parameter("wqT", [128, HPC, HCH, D], BF16, False)
    woT = nc.declare_dram_parameter("woT", [128, NH, H], BF16, False)  # FULL Wo
    pkT = nc.declare_dram_parameter("pkT", [D, SP], BF16, False)
    pv = nc.declare_dram_parameter("pv", [128, SP // 128, D], BF16, False)
    cosq = nc.declare_dram_parameter("cosq", [D, SQ], BF16, False)
    sinq = nc.declare_dram_parameter("sinq", [D, SQ], BF16, False)
    out_ext = nc.declare_dram_parameter("out", [OWN, H], F32, True)

    with tile.TileContext(nc) as tc, ExitStack() as ctx:
        # ---- persistent SBUF residents
        const = ctx.enter_context(tc.tile_pool(name="const", bufs=1))
        kT_sb = const.tile([128, KV], BF16)  # roped K^T  [d, kv]
        v_sb = const.tile([128, KVCH, D], BF16)  # V chunks [kv%128, chunk, d]
        qT_sb = const.tile([128, HPC, SQ], BF16)  # roped Q^T per head [d, h, s]
        attnT_sb = const.tile([128, HPC, SQ], BF16)  # attn^T [d, h, s]
        aT_sb = const.tile([128, NCORES, HPC, OWN], BF16)  # A2A recv [d,(j,h),s]
        cos_sb = const.tile([128, SQ], BF16)  # rope tables, positions SP..KV
        sin_sb = const.tile([128, SQ], BF16)
        ident = const.tile([128, 128], BF16)
        ones_sb = const.tile([128, 128], BF16)

        make_identity(nc, ident[:, :])
        nc.vector.memset(ones_sb[:, :], 1.0)

        dram = ctx.enter_context(tc.tile_pool(name="dram", bufs=1, space="DRAM"))
        # A2A block j = rows [128j:128j+128) = [d=128, 2 heads x 128 own-q-cols]
        a2a1_in = dram.tile([NCORES * D, 2 * OWN], BF16, tag="a1i", name="a1i")
        a2a1_out = dram.tile([NCORES * D, 2 * OWN], BF16, tag="a1o", name="a1o")
        a2a2_in = dram.tile([NCORES * D, 2 * OWN], BF16, tag="a2i", name="a2i")
        a2a2_out = dram.tile([NCORES * D, 2 * OWN], BF16, tag="a2o", name="a2o")
        warm_in = dram.tile([NCORES, 128], BF16, tag="wi", name="wi")
        warm_out = dram.tile([NCORES, 128], BF16, tag="wo_", name="wo_")
        nc.gpsimd.dma_start(out=warm_in[:, :], in_=ones_sb[0:NCORES, :])
        nc.gpsimd.collective_compute(
            "AllToAll",
            mybir.AluOpType.bypass,
            ins=[warm_in[:, :].opt()],
            outs=[warm_out[:, :].opt()],
            replica_groups=[list(range(NCORES))],
        )

        rope_tmp = ctx.enter_context(tc.tile_pool(name="rope_tmp", bufs=2))

        # ================= Phase 1: projections + rope ==================
        with tc.tile_pool(name="proj", bufs=1) as proj:
            wk_sb = proj.tile([128, HCH, D], BF16)
            wv_sb = proj.tile([128, HCH, D], BF16)
            wq_sb = proj.tile([128, HPC, HCH, D], BF16)
            xT_sb = proj.tile([128, HCH, SQ], BF16)

            # DMA issue order = arrival priority. Everything is one big
            # contiguous transfer; x in quarters so the c-loop can chase it.
            nc.sync.dma_start(out=wk_sb[:, :, :], in_=wkT[:, :, :])
            nc.sync.dma_start(out=wv_sb[:, :, :], in_=wvT[:, :, :])
            nc.sync.dma_start(out=wq_sb[:, 0, :, :], in_=wqT[:, 0, :, :])
            for qtr in range(4):
                nc.sync.dma_start(
                    out=xT_sb[:, ds(qtr * 8, 8), :], in_=xT[:, ds(qtr * 8, 8), :]
                )
            nc.sync.dma_start(out=cos_sb[:, :], in_=cosq[:, :])
            nc.sync.dma_start(out=sin_sb[:, :], in_=sinq[:, :])
            for j in range(1, HPC):
                nc.sync.dma_start(out=wq_sb[:, j, :, :], in_=wqT[:, j, :, :])
            nc.sync.dma_start(out=kT_sb[:, 0:SP], in_=pkT[:, :])
            nc.sync.dma_start(out=v_sb[:, 0 : SP // 128, :], in_=pv[:, :, :])

            # K/V/Q-head0 projections, c-outer: 6 accumulating PSUM banks
            # pace with the x stream.
            ph1 = ctx_ph1 = ExitStack()
            kacc = ph1.enter_context(tc.tile_pool(name="kacc", bufs=2, space="PSUM"))
            vacc = ph1.enter_context(tc.tile_pool(name="vacc", bufs=2, space="PSUM"))
            qacc = ph1.enter_context(tc.tile_pool(name="qacc", bufs=2, space="PSUM"))
            tp_ps = ph1.enter_context(tc.tile_pool(name="tp_ps", bufs=2, space="PSUM"))
            k_ps = [kacc.tile([128, 512], F32, tag="k", name=f"kps{g}") for g in range(2)]
            v_ps = [vacc.tile([128, 512], F32, tag="v", name=f"vps{g}") for g in range(2)]
            q_ps = [qacc.tile([128, 512], F32, tag="q", name=f"qps{g}") for g in range(2)]
            for c in range(HCH):
                st = c == 0
                sp = c == HCH - 1
                for g in range(2):
                    nc.tensor.matmul(
                        k_ps[g][:, :],
                        lhsT=wk_sb[:, c, :],
                        rhs=xT_sb[:, c, ts(g, 512)],
                        start=st,
                        stop=sp,
                    )
                for g in range(2):
                    nc.tensor.matmul(
                        v_ps[g][:, :],
                        lhsT=wv_sb[:, c, :],
                        rhs=xT_sb[:, c, ts(g, 512)],
                        start=st,
                        stop=sp,
                    )
                for g in range(2):
                    nc.tensor.matmul(
                        q_ps[g][:, :],
                        lhsT=wq_sb[:, 0, c, :],
                        rhs=xT_sb[:, c, ts(g, 512)],
                        start=st,
                        stop=sp,
                    )
            # K_new rope -> kT_sb[:, SP:]; Q0 rope -> qT_sb[:, 0, :]
            for g in range(2):
                _rope_write(
                    nc, rope_tmp, kT_sb[:, ds(SP + g * 512, 512)], k_ps[g][:, :],
                    cos_sb, sin_sb, g * 512, 512,
                )
                _rope_write(
                    nc, rope_tmp, qT_sb[:, 0, ts(g, 512)], q_ps[g][:, :],
                    cos_sb, sin_sb, g * 512, 512,
                )
            # V^T -> transpose into v_sb chunks [SP/128 ..)
            with tc.tile_pool(name="vtmp", bufs=2) as vtmp:
                for g in range(2):
                    vt = vtmp.tile([128, 512], BF16, name="vt")
                    nc.scalar.copy(vt[:, :], v_ps[g][:, :])
                    for k in range(4):
                        ps2 = tp_ps.tile([128, 128], BF16, tag="tp", name="ps2")
                        nc.tensor.transpose(ps2[:, :], vt[:, ts(k, 128)], ident[:, :])
                        nc.scalar.copy(v_sb[:, SP // 128 + g * 4 + k, :], ps2[:, :])

            ph1.close()  # free the 8 phase-1a PSUM banks
            # Q heads 1..3, c-outer, 6 banks: no rope-WAR across heads
            with tc.tile_pool(name="qacc2", bufs=6, space="PSUM") as qacc2:
                for j in range(1, HPC):
                    q_ps = [qacc2.tile([128, 512], F32, tag="q", name=f"qps{j}{g}") for g in range(2)]
                    for c in range(HCH):
                        for g in range(2):
                            nc.tensor.matmul(
                                q_ps[g][:, :],
                                lhsT=wq_sb[:, j, c, :],
                                rhs=xT_sb[:, c, ts(g, 512)],
                                start=(c == 0),
                                stop=(c == HCH - 1),
                            )
                    for g in range(2):
                        _rope_write(
                            nc, rope_tmp, qT_sb[:, j, ts(g, 512)], q_ps[g][:, :],
                            cos_sb, sin_sb, g * 512, 512,
                        )

        # ======== Phase 2: attention + Wo prefetch + A2A; Phase 3: o_proj ====
        wo_pool = ctx.enter_context(tc.tile_pool(name="wo", bufs=4))
        pt_pool = ctx.enter_context(tc.tile_pool(name="pt", bufs=4))
        rc_pool = ctx.enter_context(tc.tile_pool(name="rc", bufs=2))
        out_pool = ctx.enter_context(tc.tile_pool(name="ob", bufs=4))

        # Wo chunk groups (4 chunks of [128, H] each) in consumption order.
        # Groups 0-3 issue now (prefetch during attention); groups 4-7 are
        # emitted on the gpsimd queue after the cc triggers so their
        # WAR-waits (on o_proj consumption) don't block the collectives.
        wo_tiles = []

        def issue_wo_group(g):
            t = wo_pool.tile([128, 4, H], BF16, tag="wo", name=f"wo{g}")
            wo_tiles.append(t)
            eng = nc.sync if g < 4 else nc.gpsimd
            for i in range(4):
                j, h = CONS[g * 4 + i]
                eng.dma_start(out=t[:, i, :], in_=woT[:, 4 * j + h, :])

        for g in range(4):
            issue_wo_group(g)

        def attention_head(h):
            for g in range(2):
                sums = sums_ps.tile([128, 512], F32, tag="sums", name=f"sums{h}{g}")
                att = at_ps.tile([128, 512], F32, tag="att", name=f"att{h}{g}")
                for c in range(KVCH):
                    st = st_ps.tile([128, 512], F32, tag="st", name="st")
                    nc.tensor.matmul(
                        st[:, :],
                        lhsT=kT_sb[:, ts(c, 128)],
                        rhs=qT_sb[:, h, ts(g, 512)],
                        start=True,
                        stop=True,
                    )
                    pt = pt_pool.tile([128, 512], BF16, name="pt")
                    nc.scalar.activation(pt[:, :], st[:, :], EXP, scale=INV_SQRT_D)
                    nc.tensor.matmul(
                        sums[:, :],
                        lhsT=ones_sb[:, :],
                        rhs=pt[:, :],
                        start=(c == 0),
                        stop=(c == KVCH - 1),
                    )
                    nc.tensor.matmul(
                        att[:, :],
                        lhsT=v_sb[:, c, :],
                        rhs=pt[:, :],
                        start=(c == 0),
                        stop=(c == KVCH - 1),
                    )
                recip = rc_pool.tile([128, 512], F32, name="recip")
                nc.vector.reciprocal_approx_fast(recip[:, :], sums[:, :])
                nc.vector.tensor_mul(
                    attnT_sb[:, h, ts(g, 512)], att[:, :], recip[:, :]
                )

        def stage_and_a2a(hs, a_in, a_out):
            # stage attnT heads hs (both q halves) as 8 dest blocks, then A2A
            for j in range(NCORES):
                nc.sync.dma_start(
                    out=a_in[ds(j * D, D), :],
                    in_=attnT_sb[:, ds(hs, 2), ds(j * OWN, OWN)],
                )
            nc.gpsimd.collective_compute(
                "AllToAll",
                mybir.AluOpType.bypass,
                ins=[a_in[:, :].opt()],
                outs=[a_out[:, :].opt()],
                replica_groups=[list(range(NCORES))],
            )
            # unpack each peer's block into aT_sb[:, j, hs:hs+2, :]
            # (on scalar: a recv blocked on cc-completion must not delay the
            # next stage's DMAs on the sync queue)
            for j in range(NCORES):
                nc.scalar.dma_start(
                    out=aT_sb[:, j, ds(hs, 2), :], in_=a_out[ds(j * D, D), :]
                )

        with tc.tile_pool(name="st_ps", bufs=4, space="PSUM") as st_ps, tc.tile_pool(
            name="sums_ps", bufs=2, space="PSUM"
        ) as sums_ps, tc.tile_pool(name="at_ps", bufs=2, space="PSUM") as at_ps:
            attention_head(0)
            attention_head(1)
            stage_and_a2a(0, a2a1_in, a2a1_out)
            attention_head(2)
            attention_head(3)
            stage_and_a2a(2, a2a2_in, a2a2_out)

        # ---- o_proj: own 128 rows, full 4096 contraction, 8 PSUM banks
        with tc.tile_pool(name="ob_ps", bufs=1, space="PSUM") as ob_ps:
            o_ps = [
                ob_ps.tile([128, 512], F32, tag=f"ob{n}", name=f"ob{n}")
                for n in range(H // 512)
            ]
            for k, (j, h) in enumerate(CONS):
                if k % 4 == 0 and k // 4 >= 4:
                    issue_wo_group(k // 4)
                wo_t = wo_tiles[k // 4]
                for n in range(H // 512):
                    nc.tensor.matmul(
                        o_ps[n][:, :],
                        lhsT=aT_sb[:, j, h, :],
                        rhs=wo_t[:, k % 4, ts(n, 512)],
                        start=(k == 0),
                        stop=(k == len(CONS) - 1),
                    )
            for n in range(H // 512):
                ob = out_pool.tile([128, 512], F32, tag="ob", name="ob")
                nc.vector.tensor_copy(ob[:, :], o_ps[n][:, :])
                nc.sync.dma_start(out=out_ext[:, ts(n, 512)], in_=ob[:, :])

    nc.finalize()
    return nc


def _get_nc():
    if "nc" not in _NC_CACHE:
        _NC_CACHE["nc"] = _build_nc()
    return _NC_CACHE["nc"]


def _rope_tables():
    inv_freq = 1.0 / (ROPE_THETA ** (np.arange(0, D, 2, dtype=np.float32) / D))
    pos = np.arange(KV, dtype=np.float32)
    freqs = pos[:, None] * inv_freq[None, :]  # [KV, D/2]
    emb = np.concatenate([freqs, freqs], axis=-1)  # [KV, D]
    return np.cos(emb), np.sin(emb)  # [KV, D]


def _host_rope(x, cos, sin):
    # x: [S, D]; cos/sin: [S, D]
    x1, x2 = x[:, : D // 2], x[:, D // 2 :]
    rot = np.concatenate([-x2, x1], axis=-1)
    return x * cos + rot * sin


def _pack_chunks(a):
    """[N*128, F] -> [128, N, F] with [p, c, f] = a[128c+p, f]."""
    n = a.shape[0] // 128
    return np.ascontiguousarray(a.reshape(n, 128, -1).transpose(1, 0, 2))


def kernel(hidden_states, past_k, past_v, Wq, Wk, Wv, Wo, trace=False):
    global LAST_RESULT
    bf = ml_dtypes.bfloat16
    x = np.asarray(hidden_states, dtype=np.float32)[0]  # [SQ, H]
    xT_p = _pack_chunks(np.ascontiguousarray(x.T)).astype(bf)  # [128, 32, 1024]
    cos, sin = _rope_tables()  # [KV, D] f32
    cosq = np.ascontiguousarray(cos[SP:].T).astype(bf)  # [128, 1024]
    sinq = np.ascontiguousarray(sin[SP:].T).astype(bf)
    woT_p = _pack_chunks(np.ascontiguousarray(np.asarray(Wo, dtype=np.float32).T)).astype(
        bf
    )  # [128, 32, 4096], full Wo, replicated

    Wq_n = np.asarray(Wq, dtype=np.float32)
    Wk_n = np.asarray(Wk, dtype=np.float32)
    Wv_n = np.asarray(Wv, dtype=np.float32)
    pk_n = np.asarray(past_k, dtype=np.float32)
    pv_n = np.asarray(past_v, dtype=np.float32)

    in_maps = []
    for m in range(NCORES):
        kr = slice(m * D, (m + 1) * D)
        wq_j = np.stack(
            [
                _pack_chunks(np.ascontiguousarray(Wq_n[m * DQ + j * D : m * DQ + (j + 1) * D].T))
                for j in range(HPC)
            ],
            axis=1,
        )  # [128, HPC, HCH, D]
        in_maps.append(
            {
                "xT": xT_p,
                "wkT": _pack_chunks(np.ascontiguousarray(Wk_n[kr].T)).astype(bf),
                "wvT": _pack_chunks(np.ascontiguousarray(Wv_n[kr].T)).astype(bf),
                "wqT": np.ascontiguousarray(wq_j).astype(bf),
                "woT": woT_p,
                "pkT": np.ascontiguousarray(
                    _host_rope(pk_n[0, m], cos[:SP], sin[:SP]).T
                ).astype(bf),
                "pv": _pack_chunks(np.ascontiguousarray(pv_n[0, m])).astype(bf),
                "cosq": cosq,
                "sinq": sinq,
            }
        )

    nc = _get_nc()
    res = run_bass_kernel_spmd(
        nc, in_maps, core_ids=list(range(NCORES)), trace=trace
    )
    LAST_RESULT = res
    out = np.empty((SQ, H), dtype=np.float32)
    for m in range(NCORES):
        out[m * OWN : (m + 1) * OWN] = np.asarray(
            res.results[m]["out"], dtype=np.float32
        )
    return out.reshape(B, SQ, H)


# revision 12
# speedup vs baseline: 1.0421x; 1.0421x over previous
"""Tensor-parallel GQA attention block (AtlasAttentionWrapper) on 8 TRN2 cores.

Sharding: TP over heads. Core m owns query heads [4m..4m+3] (Wq rows
m*512:(m+1)*512) and KV head m (Wk/Wv rows m*128:(m+1)*128, past_k/past_v
head m).

o_proj strategy (v2): instead of ReduceScattering the 8MB o_proj partials
(CC-serialized ~128us, ~60us exposed tail in v1), the cores AllToAll the
*attention outputs* (2 x 512KB total) and each core computes the o_proj for
its own 128 output rows with the FULL 4096-dim contraction, streaming the
full Wo (32MB, replicated input) through SBUF. Comm drops 8x and is fully
hidden; core m returns final rows [128m:128(m+1)) (host concatenates).

Host pre-packs every input so each tensor is one (or a few) large contiguous
DMAs ([128, ...] partition-major): v1 spent ~70us serializing ~211 small DMA
descriptor issues at startup.

Phase 1 is c-outer: per x-chunk c, the K/V/Q(head0) projections each issue
their 2 accumulating matmuls (6 PSUM banks), so the PE starts ~3us in and
paces with the x DMA stream instead of waiting for full tensors.

All matmuls bf16 with f32 PSUM accumulation. Scores are built transposed
(S^T[kv, q] = K Q^T) so the exp() lands in the [kv, q] layout the PV matmul
consumes; the softmax denominator comes from a ones-row matmul; the 1/sum
normalization is fused into the PSUM->SBUF copy of attn^T. No
max-subtraction: score scale is ~N(0, 1.7), exp() is safe in f32/bf16.
"""

import sys

if "/opt/trn_rl_repo" not in sys.path:
    sys.path.insert(0, "/opt/trn_rl_repo")

from contextlib import ExitStack

import ml_dtypes
import numpy as np

import concourse.bass as bass
import concourse.tile as tile
from concourse import bacc, mybir
from concourse.bass import ds, ts
from concourse.bass_utils import run_bass_kernel_spmd
from concourse.masks import make_identity

NCORES = 8
B, SQ, H = 1, 1024, 4096
NH, NKV, D = 32, 8, 128
SP = 1024
KV = SP + SQ  # 2048
HPC = NH // NCORES  # 4 query heads per core
DQ = HPC * D  # 512
OWN = SQ // NCORES  # 128 output rows owned per core
ROPE_THETA = 10000.0
INV_SQRT_D = 1.0 / float(np.sqrt(D))

BF16 = mybir.dt.bfloat16
F32 = mybir.dt.float32
HCH = H // 128  # 32 contraction chunks for the projections
KVCH = KV // 128  # 16 kv chunks
EXP = mybir.ActivationFunctionType.Exp

LAST_RESULT = None
_NC_CACHE = {}

# o_proj contraction-chunk consumption order: global head chunk 4*j+h for
# (core j, local head h). A2A#1 carries heads {0,1}, A2A#2 heads {2,3};
# consume #1's 16 chunks first so A2A#2 hides under them.
CONS = [(j, h) for h in (0, 1) for j in range(NCORES)] + [
    (j, h) for h in (2, 3) for j in range(NCORES)
]
WO_GROUPS = 8  # 4 chunks per group, one wo pool buf each


def _rope_write(nc, tmp_pool, dst, src, cos_sb, sin_sb, pos, width):
    """dst[d, s] = rope(src)[d, s]; cos/sin tables indexed at pos (table-rel).

    src: AP [128, width] (PSUM f32), dst: SBUF bf16 AP.
    rope: out[d<64] = x[d]*cos[d] - x[d+64]*sin[d]
          out[d>=64] = x[d]*cos[d] + x[d-64]*sin[d]
    """
    cs = cos_sb[:, ds(pos, width)]
    sn = sin_sb[:, ds(pos, width)]
    t = tmp_pool.tile([128, width], F32, tag="rope_t")
    u = tmp_pool.tile([128, width], F32, tag="rope_u")
    nc.vector.tensor_mul(t[0:64, :], src[64:128, :], sn[0:64, :])
    nc.vector.tensor_mul(t[64:128, :], src[0:64, :], sn[64:128, :])
    nc.vector.tensor_mul(u[:, :], src[:, :], cs)
    nc.vector.tensor_sub(dst[0:64, :], u[0:64, :], t[0:64, :])
    nc.vector.tensor_add(dst[64:128, :], u[64:128, :], t[64:128, :])


def _build_nc():
    nc = bacc.Bacc(None, target_bir_lowering=False, debug=False)

    # Host-packed DRAM inputs (partition-major, one contiguous DMA each).
    xT = nc.declare_dram_parameter("xT", [128, HCH, SQ], BF16, False)
    wkT = nc.declare_dram_parameter("wkT", [128, HCH, D], BF16, False)
    wvT = nc.declare_dram_parameter("wvT", [128, HCH, D], BF16, False)
    wqT = nc.declare_dram_parameter("wqT", [128, HPC, HCH, D], BF16, False)
    woT = nc.declare_dram_parameter("woT", [128, NH, H], BF16, False)  # FULL Wo
    pkT = nc.declare_dram_parameter("pkT", [D, SP], BF16, False)
    pv = nc.declare_dram_parameter("pv", [128, SP // 128, D], BF16, False)
    cosq = nc.declare_dram_parameter("cosq", [D, SQ], BF16, False)
    sinq = nc.declare_dram_parameter("sinq", [D, SQ], BF16, False)
    out_ext = nc.declare_dram_parameter("out", [OWN, H], F32, True)

    with tile.TileContext(nc) as tc, ExitStack() as ctx:
        # ---- persistent SBUF residents
        const = ctx.enter_context(tc.tile_pool(name="const", bufs=1))
        kT_sb = const.tile([128, KV], BF16)  # roped K^T  [d, kv]
        v_sb = const.tile([128, KVCH, D], BF16)  # V chunks [kv%128, chunk, d]
        qT_sb = const.tile([128, HPC, SQ], BF16)  # roped Q^T per head [d, h, s]
        attnT_sb = const.tile([128, HPC, SQ], BF16)  # attn^T [d, h, s]
        aT_sb = const.tile([128, NCORES, HPC, OWN], BF16)  # A2A recv [d,(j,h),s]
        cos_sb = const.tile([128, SQ], BF16)  # rope tables, positions SP..KV
        sin_sb = const.tile([128, SQ], BF16)
        ident = const.tile([128, 128], BF16)
        ones_sb = const.tile([128, 128], BF16)

        make_identity(nc, ident[:, :])
        nc.vector.memset(ones_sb[:, :], 1.0)

        dram = ctx.enter_context(tc.tile_pool(name="dram", bufs=1, space="DRAM"))
        # A2A block j = rows [128j:128j+128) = [d=128, 2 heads x 128 own-q-cols]
        a2a1_in = dram.tile([NCORES * D, 2 * OWN], BF16, tag="a1i", name="a1i")
        a2a1_out = dram.tile([NCORES * D, 2 * OWN], BF16, tag="a1o", name="a1o")
        a2a2_in = dram.tile([NCORES * D, 2 * OWN], BF16, tag="a2i", name="a2i")
        a2a2_out = dram.tile([NCORES * D, 2 * OWN], BF16, tag="a2o", name="a2o")

        rope_tmp = ctx.enter_context(tc.tile_pool(name="rope_tmp", bufs=2))
        cp_pool = ctx.enter_context(tc.tile_pool(name="cp", bufs=4))

        # ================= Phase 1: projections + rope ==================
        with tc.tile_pool(name="proj", bufs=1) as proj:
            wk_sb = proj.tile([128, HCH, D], BF16)
            wv_sb = proj.tile([128, HCH, D], BF16)
            wq_sb = proj.tile([128, HPC, HCH, D], BF16)
            xT_sb = proj.tile([128, HCH, SQ], BF16)

            # DMA issue order = arrival priority. Everything is one big
            # contiguous transfer; x in quarters so the c-loop can chase it.
            nc.sync.dma_start(out=wk_sb[:, :, :], in_=wkT[:, :, :])
            nc.sync.dma_start(out=wv_sb[:, :, :], in_=wvT[:, :, :])
            nc.sync.dma_start(out=wq_sb[:, 0, :, :], in_=wqT[:, 0, :, :])
            for qtr in range(4):
                nc.sync.dma_start(
                    out=xT_sb[:, ds(qtr * 8, 8), :], in_=xT[:, ds(qtr * 8, 8), :]
                )
            nc.sync.dma_start(out=cos_sb[:, :], in_=cosq[:, :])
            nc.sync.dma_start(out=sin_sb[:, :], in_=sinq[:, :])
            for j in range(1, HPC):
                nc.sync.dma_start(out=wq_sb[:, j, :, :], in_=wqT[:, j, :, :])
            nc.sync.dma_start(out=kT_sb[:, 0:SP], in_=pkT[:, :])
            nc.sync.dma_start(out=v_sb[:, 0 : SP // 128, :], in_=pv[:, :, :])

            # K/V/Q-head0 projections, c-outer: 6 accumulating PSUM banks
            # pace with the x stream.
            ph1 = ctx_ph1 = ExitStack()
            kacc = ph1.enter_context(tc.tile_pool(name="kacc", bufs=2, space="PSUM"))
            vacc = ph1.enter_context(tc.tile_pool(name="vacc", bufs=2, space="PSUM"))
            qacc = ph1.enter_context(tc.tile_pool(name="qacc", bufs=2, space="PSUM"))
            tp_ps = ph1.enter_context(tc.tile_pool(name="tp_ps", bufs=2, space="PSUM"))
            k_ps = [kacc.tile([128, 512], F32, tag="k", name=f"kps{g}") for g in range(2)]
            v_ps = [vacc.tile([128, 512], F32, tag="v", name=f"vps{g}") for g in range(2)]
            q_ps = [qacc.tile([128, 512], F32, tag="q", name=f"qps{g}") for g in range(2)]
            for c in range(HCH):
                st = c == 0
                sp = c == HCH - 1
                for g in range(2):
                    nc.tensor.matmul(
                        k_ps[g][:, :],
                        lhsT=wk_sb[:, c, :],
                        rhs=xT_sb[:, c, ts(g, 512)],
                        start=st,
                        stop=sp,
                    )
                for g in range(2):
                    nc.tensor.matmul(
                        v_ps[g][:, :],
                        lhsT=wv_sb[:, c, :],
                        rhs=xT_sb[:, c, ts(g, 512)],
                        start=st,
                        stop=sp,
                    )
                for g in range(2):
                    nc.tensor.matmul(
                        q_ps[g][:, :],
                        lhsT=wq_sb[:, 0, c, :],
                        rhs=xT_sb[:, c, ts(g, 512)],
                        start=st,
                        stop=sp,
                    )
            # V^T -> transpose into v_sb chunks [SP/128 ..)  (vt copies first
            # on scalar: the PE transposes gate on them)
            with tc.tile_pool(name="vtmp", bufs=2) as vtmp:
                for g in range(2):
                    vt = vtmp.tile([128, 512], BF16, name="vt")
                    nc.scalar.copy(vt[:, :], v_ps[g][:, :])
                    for k in range(4):
                        ps2 = tp_ps.tile([128, 128], BF16, tag="tp", name="ps2")
                        nc.tensor.transpose(ps2[:, :], vt[:, ts(k, 128)], ident[:, :])
                        nc.scalar.copy(v_sb[:, SP // 128 + g * 4 + k, :], ps2[:, :])

            # Release the k/q accumulator banks fast (scalar copies: straight
            # + half-swapped), so the Qj1-3 matmuls reusing those banks don't
            # wait on the serial DVE ropes; ropes then read SBUF.
            kq_cp = []
            for nm, ps_pair in (("k", k_ps), ("q", q_ps)):
                for g in range(2):
                    cp = cp_pool.tile([128, 512], F32, tag="cp", name=f"{nm}cp{g}")
                    sw = cp_pool.tile([128, 512], F32, tag="sw", name=f"{nm}sw{g}")
                    nc.scalar.copy(cp[:, :], ps_pair[g][:, :])
                    nc.scalar.copy(sw[0:64, :], ps_pair[g][64:128, :])
                    nc.scalar.copy(sw[64:128, :], ps_pair[g][0:64, :])
                    kq_cp.append((cp, sw))

            def rope_sb(dst, cp_sw, pos):
                cp, sw = cp_sw
                cs = cos_sb[:, ds(pos, 512)]
                sn = sin_sb[:, ds(pos, 512)]
                t = rope_tmp.tile([128, 512], F32, tag="rope_t", name="t")
                u = rope_tmp.tile([128, 512], F32, tag="rope_u", name="u")
                nc.vector.tensor_mul(t[:, :], sw[:, :], sn)
                nc.vector.tensor_mul(u[:, :], cp[:, :], cs)
                nc.vector.tensor_sub(dst[0:64, :], u[0:64, :], t[0:64, :])
                nc.vector.tensor_add(dst[64:128, :], u[64:128, :], t[64:128, :])

            for g in range(2):
                rope_sb(kT_sb[:, ds(SP + g * 512, 512)], kq_cp[g], g * 512)
            for g in range(2):
                rope_sb(qT_sb[:, 0, ts(g, 512)], kq_cp[2 + g], g * 512)

            ph1.close()  # free the 8 phase-1a PSUM banks
            # Q heads 1..3, c-outer, 6 banks: no rope-WAR across heads
            with tc.tile_pool(name="qacc2", bufs=6, space="PSUM") as qacc2:
                for j in range(1, HPC):
                    q_ps = [qacc2.tile([128, 512], F32, tag="q", name=f"qps{j}{g}") for g in range(2)]
                    for c in range(HCH):
                        for g in range(2):
                            nc.tensor.matmul(
                                q_ps[g][:, :],
                                lhsT=wq_sb[:, j, c, :],
                                rhs=xT_sb[:, c, ts(g, 512)],
                                start=(c == 0),
                                stop=(c == HCH - 1),
                            )
                    for g in range(2):
                        _rope_write(
                            nc, rope_tmp, qT_sb[:, j, ts(g, 512)], q_ps[g][:, :],
                            cos_sb, sin_sb, g * 512, 512,
                        )

        # ======== Phase 2: attention + Wo prefetch + A2A; Phase 3: o_proj ====
        wo_pool = ctx.enter_context(tc.tile_pool(name="wo", bufs=4))
        pt_pool = ctx.enter_context(tc.tile_pool(name="pt", bufs=4))
        rc_pool = ctx.enter_context(tc.tile_pool(name="rc", bufs=2))
        out_pool = ctx.enter_context(tc.tile_pool(name="ob", bufs=4))

        # Wo chunk groups (4 chunks of [128, H] each) in consumption order.
        # Groups 0-3 issue now (prefetch during attention); groups 4-7 are
        # emitted on the gpsimd queue after the cc triggers so their
        # WAR-waits (on o_proj consumption) don't block the collectives.
        wo_tiles = []

        def issue_wo_group(g):
            t = wo_pool.tile([128, 4, H], BF16, tag="wo", name=f"wo{g}")
            wo_tiles.append(t)
            eng = nc.sync if g < 4 else nc.gpsimd
            for i in range(4):
                j, h = CONS[g * 4 + i]
                eng.dma_start(out=t[:, i, :], in_=woT[:, 4 * j + h, :])

        for g in range(4):
            issue_wo_group(g)

        def attention_head(h):
            for g in range(2):
                sums = sums_ps.tile([128, 512], F32, tag="sums", name=f"sums{h}{g}")
                att = at_ps.tile([128, 512], F32, tag="att", name=f"att{h}{g}")
                for c in range(KVCH):
                    st = st_ps.tile([128, 512], F32, tag="st", name="st")
                    nc.tensor.matmul(
                        st[:, :],
                        lhsT=kT_sb[:, ts(c, 128)],
                        rhs=qT_sb[:, h, ts(g, 512)],
                        start=True,
                        stop=True,
                    )
                    pt = pt_pool.tile([128, 512], BF16, name="pt")
                    nc.scalar.activation(pt[:, :], st[:, :], EXP, scale=INV_SQRT_D)
                    nc.tensor.matmul(
                        sums[:, :],
                        lhsT=ones_sb[:, :],
                        rhs=pt[:, :],
                        start=(c == 0),
                        stop=(c == KVCH - 1),
                    )
                    nc.tensor.matmul(
                        att[:, :],
                        lhsT=v_sb[:, c, :],
                        rhs=pt[:, :],
                        start=(c == 0),
                        stop=(c == KVCH - 1),
                    )
                recip = rc_pool.tile([128, 512], F32, name="recip")
                nc.vector.reciprocal_approx_fast(recip[:, :], sums[:, :])
                nc.vector.tensor_mul(
                    attnT_sb[:, h, ts(g, 512)], att[:, :], recip[:, :]
                )

        def stage_and_a2a(hs, a_in, a_out):
            # stage attnT heads hs (both q halves) as 8 dest blocks, then A2A
            for j in range(NCORES):
                nc.sync.dma_start(
                    out=a_in[ds(j * D, D), :],
                    in_=attnT_sb[:, ds(hs, 2), ds(j * OWN, OWN)],
                )
            nc.gpsimd.collective_compute(
                "AllToAll",
                mybir.AluOpType.bypass,
                ins=[a_in[:, :].opt()],
                outs=[a_out[:, :].opt()],
                replica_groups=[list(range(NCORES))],
            )


        with tc.tile_pool(name="st_ps", bufs=4, space="PSUM") as st_ps, tc.tile_pool(
            name="sums_ps", bufs=2, space="PSUM"
        ) as sums_ps, tc.tile_pool(name="at_ps", bufs=2, space="PSUM") as at_ps:
            attention_head(0)
            attention_head(1)
            stage_and_a2a(0, a2a1_in, a2a1_out)
            attention_head(2)
            attention_head(3)
            stage_and_a2a(2, a2a2_in, a2a2_out)
            # recv unpack AFTER all exp emission (scalar queue: a recv
            # wait on cc completion must not starve the h2/h3 exps)
            for hs, a_out in ((0, a2a1_out), (2, a2a2_out)):
                for j in range(NCORES):
                    nc.scalar.dma_start(
                        out=aT_sb[:, j, ds(hs, 2), :], in_=a_out[ds(j * D, D), :]
                    )

        # ---- o_proj: own 128 rows, full 4096 contraction, 8 PSUM banks
        with tc.tile_pool(name="ob_ps", bufs=1, space="PSUM") as ob_ps:
            o_ps = [
                ob_ps.tile([128, 512], F32, tag=f"ob{n}", name=f"ob{n}")
                for n in range(H // 512)
            ]
            for k, (j, h) in enumerate(CONS):
                if k % 4 == 0 and k // 4 >= 4:
                    issue_wo_group(k // 4)
                wo_t = wo_tiles[k // 4]
                for n in range(H // 512):
                    nc.tensor.matmul(
                        o_ps[n][:, :],
                        lhsT=aT_sb[:, j, h, :],
                        rhs=wo_t[:, k % 4, ts(n, 512)],
                        start=(k == 0),
                        stop=(k == len(CONS) - 1),
                    )
            for n in range(H // 512):
                ob = out_pool.tile([128, 512], F32, tag="ob", name="ob")
                nc.vector.tensor_copy(ob[:, :], o_ps[n][:, :])
                nc.sync.dma_start(out=out_ext[:, ts(n, 512)], in_=ob[:, :])

    nc.finalize()
    return nc


def _get_nc():
    if "nc" not in _NC_CACHE:
        _NC_CACHE["nc"] = _build_nc()
    return _NC_CACHE["nc"]


def _rope_tables():
    inv_freq = 1.0 / (ROPE_THETA ** (np.arange(0, D, 2, dtype=np.float32) / D))
    pos = np.arange(KV, dtype=np.float32)
    freqs = pos[:, None] * inv_freq[None, :]  # [KV, D/2]
    emb = np.concatenate([freqs, freqs], axis=-1)  # [KV, D]
    return np.cos(emb), np.sin(emb)  # [KV, D]


def _host_rope(x, cos, sin):
    # x: [S, D]; cos/sin: [S, D]
    x1, x2 = x[:, : D // 2], x[:, D // 2 :]
    rot = np.concatenate([-x2, x1], axis=-1)
    return x * cos + rot * sin


def _pack_chunks(a):
    """[N*128, F] -> [128, N, F] with [p, c, f] = a[128c+p, f]."""
    n = a.shape[0] // 128
    return np.ascontiguousarray(a.reshape(n, 128, -1).transpose(1, 0, 2))


def kernel(hidden_states, past_k, past_v, Wq, Wk, Wv, Wo, trace=False):
    global LAST_RESULT
    bf = ml_dtypes.bfloat16
    x = np.asarray(hidden_states, dtype=np.float32)[0]  # [SQ, H]
    xT_p = _pack_chunks(np.ascontiguousarray(x.T)).astype(bf)  # [128, 32, 1024]
    cos, sin = _rope_tables()  # [KV, D] f32
    cosq = np.ascontiguousarray(cos[SP:].T).astype(bf)  # [128, 1024]
    sinq = np.ascontiguousarray(sin[SP:].T).astype(bf)
    woT_p = _pack_chunks(np.ascontiguousarray(np.asarray(Wo, dtype=np.float32).T)).astype(
        bf
    )  # [128, 32, 4096], full Wo, replicated

    Wq_n = np.asarray(Wq, dtype=np.float32)
    Wk_n = np.asarray(Wk, dtype=np.float32)
    Wv_n = np.asarray(Wv, dtype=np.float32)
    pk_n = np.asarray(past_k, dtype=np.float32)
    pv_n = np.asarray(past_v, dtype=np.float32)

    in_maps = []
    for m in range(NCORES):
        kr = slice(m * D, (m + 1) * D)
        wq_j = np.stack(
            [
                _pack_chunks(np.ascontiguousarray(Wq_n[m * DQ + j * D : m * DQ + (j + 1) * D].T))
                for j in range(HPC)
            ],
            axis=1,
        )  # [128, HPC, HCH, D]
        in_maps.append(
            {
                "xT": xT_p,
                "wkT": _pack_chunks(np.ascontiguousarray(Wk_n[kr].T)).astype(bf),
                "wvT": _pack_chunks(np.ascontiguousarray(Wv_n[kr].T)).astype(bf),
                "wqT": np.ascontiguousarray(wq_j).astype(bf),
                "woT": woT_p,
                "pkT": np.ascontiguousarray(
                    _host_rope(pk_n[0, m], cos[:SP], sin[:SP]).T
                ).astype(bf),
                "pv": _pack_chunks(np.ascontiguousarray(pv_n[0, m])).astype(bf),
                "cosq": cosq,
                "sinq": sinq,
            }
        )

    nc = _get_nc()
    res = run_bass_kernel_spmd(
        nc, in_maps, core_ids=list(range(NCORES)), trace=trace
    )
    LAST_RESULT = res
    out = np.empty((SQ, H), dtype=np.float32)
    for m in range(NCORES):
        out[m * OWN : (m + 1) * OWN] = np.asarray(
            res.results[m]["out"], dtype=np.float32
        )
    return out.reshape(B, SQ, H)


# revision 14
# speedup vs baseline: 1.1519x; 1.1054x over previous
"""Tensor-parallel GQA attention block (AtlasAttentionWrapper) on 8 TRN2 cores.

Sharding: TP over heads. Core m owns query heads [4m..4m+3] (Wq rows
m*512:(m+1)*512) and KV head m (Wk/Wv rows m*128:(m+1)*128, past_k/past_v
head m).

o_proj strategy (v2): instead of ReduceScattering the 8MB o_proj partials
(CC-serialized ~128us, ~60us exposed tail in v1), the cores AllToAll the
*attention outputs* (2 x 512KB total) and each core computes the o_proj for
its own 128 output rows with the FULL 4096-dim contraction, streaming the
full Wo (32MB, replicated input) through SBUF. Comm drops 8x and is fully
hidden; core m returns final rows [128m:128(m+1)) (host concatenates).

Host pre-packs every input so each tensor is one (or a few) large contiguous
DMAs ([128, ...] partition-major): v1 spent ~70us serializing ~211 small DMA
descriptor issues at startup.

Phase 1 is c-outer: per x-chunk c, the K/V/Q(head0) projections each issue
their 2 accumulating matmuls (6 PSUM banks), so the PE starts ~3us in and
paces with the x DMA stream instead of waiting for full tensors.

All matmuls bf16 with f32 PSUM accumulation. Scores are built transposed
(S^T[kv, q] = K Q^T) so the exp() lands in the [kv, q] layout the PV matmul
consumes; the softmax denominator comes from a ones-row matmul; the 1/sum
normalization is fused into the PSUM->SBUF copy of attn^T. No
max-subtraction: score scale is ~N(0, 1.7), exp() is safe in f32/bf16.
"""

import sys

if "/opt/trn_rl_repo" not in sys.path:
    sys.path.insert(0, "/opt/trn_rl_repo")

from contextlib import ExitStack

import ml_dtypes
import numpy as np

import concourse.bass as bass
import concourse.tile as tile
from concourse import bacc, mybir
from concourse.bass import ds, ts
from concourse.bass_utils import run_bass_kernel_spmd
from concourse.masks import make_identity

NCORES = 8
B, SQ, H = 1, 1024, 4096
NH, NKV, D = 32, 8, 128
SP = 1024
KV = SP + SQ  # 2048
HPC = NH // NCORES  # 4 query heads per core
DQ = HPC * D  # 512
OWN = SQ // NCORES  # 128 output rows owned per core
ROPE_THETA = 10000.0
INV_SQRT_D = 1.0 / float(np.sqrt(D))

BF16 = mybir.dt.bfloat16
F32 = mybir.dt.float32
HCH = H // 128  # 32 contraction chunks for the projections
KVCH = KV // 128  # 16 kv chunks
EXP = mybir.ActivationFunctionType.Exp

LAST_RESULT = None
_NC_CACHE = {}

# o_proj contraction-chunk consumption order: global head chunk 4*j+h for
# (core j, local head h). A2A#1 carries heads {0,1}, A2A#2 heads {2,3};
# consume #1's 16 chunks first so A2A#2 hides under them.
CONS = [(j, h) for h in (0, 1) for j in range(NCORES)] + [
    (j, h) for h in (2, 3) for j in range(NCORES)
]
WO_GROUPS = 8  # 4 chunks per group, one wo pool buf each


def _rope_write(nc, tmp_pool, dst, src, cos_sb, sin_sb, pos, width):
    """dst[d, s] = rope(src)[d, s]; cos/sin tables indexed at pos (table-rel).

    src: AP [128, width] (PSUM f32), dst: SBUF bf16 AP.
    rope: out[d<64] = x[d]*cos[d] - x[d+64]*sin[d]
          out[d>=64] = x[d]*cos[d] + x[d-64]*sin[d]
    """
    cs = cos_sb[:, ds(pos, width)]
    sn = sin_sb[:, ds(pos, width)]
    t = tmp_pool.tile([128, width], F32, tag="rope_t")
    u = tmp_pool.tile([128, width], F32, tag="rope_u")
    nc.vector.tensor_mul(t[0:64, :], src[64:128, :], sn[0:64, :])
    nc.vector.tensor_mul(t[64:128, :], src[0:64, :], sn[64:128, :])
    nc.vector.tensor_mul(u[:, :], src[:, :], cs)
    nc.vector.tensor_sub(dst[0:64, :], u[0:64, :], t[0:64, :])
    nc.vector.tensor_add(dst[64:128, :], u[64:128, :], t[64:128, :])


def _build_nc():
    nc = bacc.Bacc(None, target_bir_lowering=False, debug=False)

    # Host-packed DRAM inputs (partition-major, one contiguous DMA each).
    xT = nc.declare_dram_parameter("xT", [128, HCH, SQ], BF16, False)
    wkT = nc.declare_dram_parameter("wkT", [128, HCH, D], BF16, False)
    wvT = nc.declare_dram_parameter("wvT", [128, HCH, D], BF16, False)
    wqT = nc.declare_dram_parameter("wqT", [128, HPC, HCH, D], BF16, False)
    woT = nc.declare_dram_parameter("woT", [128, NH, H], BF16, False)  # FULL Wo
    pkT = nc.declare_dram_parameter("pkT", [D, SP], BF16, False)
    pv = nc.declare_dram_parameter("pv", [128, SP // 128, D], BF16, False)
    cosq = nc.declare_dram_parameter("cosq", [D, SQ], BF16, False)
    sinq = nc.declare_dram_parameter("sinq", [D, SQ], BF16, False)
    out_ext = nc.declare_dram_parameter("out", [OWN, H], F32, True)

    with tile.TileContext(nc) as tc, ExitStack() as ctx:
        # ---- persistent SBUF residents
        const = ctx.enter_context(tc.tile_pool(name="const", bufs=1))
        kT_sb = const.tile([128, KV], BF16)  # roped K^T  [d, kv]
        v_sb = const.tile([128, KVCH, D], BF16)  # V chunks [kv%128, chunk, d]
        qT_sb = const.tile([128, HPC, SQ], BF16)  # roped Q^T per head [d, h, s]
        attnT_sb = const.tile([128, HPC, SQ], BF16)  # attn^T [d, h, s]
        aT_sb = const.tile([128, NCORES, HPC, OWN], BF16)  # A2A recv [d,(j,h),s]
        cos_sb = const.tile([128, SQ], BF16)  # rope tables, positions SP..KV
        sin_sb = const.tile([128, SQ], BF16)
        ident = const.tile([128, 128], BF16)
        ones_sb = const.tile([128, 128], BF16)

        make_identity(nc, ident[:, :])
        nc.vector.memset(ones_sb[:, :], 1.0)

        dram = ctx.enter_context(tc.tile_pool(name="dram", bufs=1, space="DRAM"))
        # A2A block j = rows [128j:128j+128) = [d=128, 2 heads x 128 own-q-cols]
        a2a1_in = dram.tile([NCORES * D, 2 * OWN], BF16, tag="a1i", name="a1i")
        a2a1_out = dram.tile([NCORES * D, 2 * OWN], BF16, tag="a1o", name="a1o")
        a2a2_in = dram.tile([NCORES * D, OWN], BF16, tag="a2i", name="a2i")
        a2a2_out = dram.tile([NCORES * D, OWN], BF16, tag="a2o", name="a2o")
        a2a3_in = dram.tile([NCORES * D, OWN], BF16, tag="a3i", name="a3i")
        a2a3_out = dram.tile([NCORES * D, OWN], BF16, tag="a3o", name="a3o")

        rope_tmp = ctx.enter_context(tc.tile_pool(name="rope_tmp", bufs=2))
        cp_pool = ctx.enter_context(tc.tile_pool(name="cp", bufs=4))

        # ================= Phase 1: projections + rope ==================
        with tc.tile_pool(name="proj", bufs=1) as proj:
            wk_sb = proj.tile([128, HCH, D], BF16)
            wv_sb = proj.tile([128, HCH, D], BF16)
            wq_sb = proj.tile([128, HPC, HCH, D], BF16)
            xT_sb = proj.tile([128, HCH, SQ], BF16)

            # DMA issue order = arrival priority. Everything is one big
            # contiguous transfer; x in quarters so the c-loop can chase it.
            nc.sync.dma_start(out=wk_sb[:, :, :], in_=wkT[:, :, :])
            nc.sync.dma_start(out=wv_sb[:, :, :], in_=wvT[:, :, :])
            nc.sync.dma_start(out=wq_sb[:, 0, :, :], in_=wqT[:, 0, :, :])
            for qtr in range(4):
                nc.sync.dma_start(
                    out=xT_sb[:, ds(qtr * 8, 8), :], in_=xT[:, ds(qtr * 8, 8), :]
                )
            nc.sync.dma_start(out=cos_sb[:, :], in_=cosq[:, :])
            nc.sync.dma_start(out=sin_sb[:, :], in_=sinq[:, :])
            for j in range(1, HPC):
                nc.sync.dma_start(out=wq_sb[:, j, :, :], in_=wqT[:, j, :, :])
            nc.sync.dma_start(out=kT_sb[:, 0:SP], in_=pkT[:, :])
            nc.sync.dma_start(out=v_sb[:, 0 : SP // 128, :], in_=pv[:, :, :])

            # K/V/Q-head0 projections, c-outer: 6 accumulating PSUM banks
            # pace with the x stream.
            ph1 = ctx_ph1 = ExitStack()
            kacc = ph1.enter_context(tc.tile_pool(name="kacc", bufs=2, space="PSUM"))
            vacc = ph1.enter_context(tc.tile_pool(name="vacc", bufs=2, space="PSUM"))
            qacc = ph1.enter_context(tc.tile_pool(name="qacc", bufs=2, space="PSUM"))
            tp_ps = ph1.enter_context(tc.tile_pool(name="tp_ps", bufs=2, space="PSUM"))
            k_ps = [kacc.tile([128, 512], F32, tag="k", name=f"kps{g}") for g in range(2)]
            v_ps = [vacc.tile([128, 512], F32, tag="v", name=f"vps{g}") for g in range(2)]
            q_ps = [qacc.tile([128, 512], F32, tag="q", name=f"qps{g}") for g in range(2)]
            for c in range(HCH):
                st = c == 0
                sp = c == HCH - 1
                for g in range(2):
                    nc.tensor.matmul(
                        k_ps[g][:, :],
                        lhsT=wk_sb[:, c, :],
                        rhs=xT_sb[:, c, ts(g, 512)],
                        start=st,
                        stop=sp,
                    )
                for g in range(2):
                    nc.tensor.matmul(
                        v_ps[g][:, :],
                        lhsT=wv_sb[:, c, :],
                        rhs=xT_sb[:, c, ts(g, 512)],
                        start=st,
                        stop=sp,
                    )
                for g in range(2):
                    nc.tensor.matmul(
                        q_ps[g][:, :],
                        lhsT=wq_sb[:, 0, c, :],
                        rhs=xT_sb[:, c, ts(g, 512)],
                        start=st,
                        stop=sp,
                    )
            # V^T -> transpose into v_sb chunks [SP/128 ..)  (vt copies first
            # on scalar: the PE transposes gate on them)
            with tc.tile_pool(name="vtmp", bufs=2) as vtmp:
                for g in range(2):
                    vt = vtmp.tile([128, 512], BF16, name="vt")
                    nc.scalar.copy(vt[:, :], v_ps[g][:, :])
                    for k in range(4):
                        ps2 = tp_ps.tile([128, 128], BF16, tag="tp", name="ps2")
                        nc.tensor.transpose(ps2[:, :], vt[:, ts(k, 128)], ident[:, :])
                        nc.scalar.copy(v_sb[:, SP // 128 + g * 4 + k, :], ps2[:, :])

            # Release the k/q accumulator banks fast (scalar copies: straight
            # + half-swapped), so the Qj1-3 matmuls reusing those banks don't
            # wait on the serial DVE ropes; ropes then read SBUF.
            kq_cp = []
            for nm, ps_pair in (("k", k_ps), ("q", q_ps)):
                for g in range(2):
                    cp = cp_pool.tile([128, 512], F32, tag="cp", name=f"{nm}cp{g}")
                    sw = cp_pool.tile([128, 512], F32, tag="sw", name=f"{nm}sw{g}")
                    nc.scalar.copy(cp[:, :], ps_pair[g][:, :])
                    nc.vector.tensor_copy(sw[0:64, :], ps_pair[g][64:128, :])
                    nc.vector.tensor_copy(sw[64:128, :], ps_pair[g][0:64, :])
                    kq_cp.append((cp, sw))

            def rope_sb(dst, cp_sw, pos):
                cp, sw = cp_sw
                cs = cos_sb[:, ds(pos, 512)]
                sn = sin_sb[:, ds(pos, 512)]
                t = rope_tmp.tile([128, 512], F32, tag="rope_t", name="t")
                u = rope_tmp.tile([128, 512], F32, tag="rope_u", name="u")
                nc.vector.tensor_mul(t[:, :], sw[:, :], sn)
                nc.vector.tensor_mul(u[:, :], cp[:, :], cs)
                nc.vector.tensor_sub(dst[0:64, :], u[0:64, :], t[0:64, :])
                nc.vector.tensor_add(dst[64:128, :], u[64:128, :], t[64:128, :])

            for g in range(2):
                rope_sb(kT_sb[:, ds(SP + g * 512, 512)], kq_cp[g], g * 512)
            for g in range(2):
                rope_sb(qT_sb[:, 0, ts(g, 512)], kq_cp[2 + g], g * 512)

            ph1.close()  # free the 8 phase-1a PSUM banks
            # Q heads 1..3, c-outer, 6 banks: no rope-WAR across heads
            with tc.tile_pool(name="qacc2", bufs=6, space="PSUM") as qacc2:
                for j in range(1, HPC):
                    q_ps = [qacc2.tile([128, 512], F32, tag="q", name=f"qps{j}{g}") for g in range(2)]
                    for c in range(HCH):
                        for g in range(2):
                            nc.tensor.matmul(
                                q_ps[g][:, :],
                                lhsT=wq_sb[:, j, c, :],
                                rhs=xT_sb[:, c, ts(g, 512)],
                                start=(c == 0),
                                stop=(c == HCH - 1),
                            )
                    for g in range(2):
                        _rope_write(
                            nc, rope_tmp, qT_sb[:, j, ts(g, 512)], q_ps[g][:, :],
                            cos_sb, sin_sb, g * 512, 512,
                        )

        # ======== Phase 2: attention + Wo prefetch + A2A; Phase 3: o_proj ====
        wo_pool = ctx.enter_context(tc.tile_pool(name="wo", bufs=4))
        pt_pool = ctx.enter_context(tc.tile_pool(name="pt", bufs=4))
        rc_pool = ctx.enter_context(tc.tile_pool(name="rc", bufs=2))
        out_pool = ctx.enter_context(tc.tile_pool(name="ob", bufs=4))

        # Wo chunk groups (4 chunks of [128, H] each) in consumption order.
        # Groups 0-3 issue now (prefetch during attention); groups 4-7 are
        # emitted on the gpsimd queue after the cc triggers so their
        # WAR-waits (on o_proj consumption) don't block the collectives.
        wo_tiles = []

        def issue_wo_group(g):
            t = wo_pool.tile([128, 4, H], BF16, tag="wo", name=f"wo{g}")
            wo_tiles.append(t)
            eng = nc.sync if g < 4 else nc.scalar
            for i in range(4):
                j, h = CONS[g * 4 + i]
                eng.dma_start(out=t[:, i, :], in_=woT[:, 4 * j + h, :])

        for g in range(4):
            issue_wo_group(g)

        def attention_head(h):
            for g in range(2):
                sums = sums_ps.tile([128, 512], F32, tag="sums", name=f"sums{h}{g}")
                att = at_ps.tile([128, 512], F32, tag="att", name=f"att{h}{g}")
                for c in range(KVCH):
                    st = st_ps.tile([128, 512], F32, tag="st", name="st")
                    nc.tensor.matmul(
                        st[:, :],
                        lhsT=kT_sb[:, ts(c, 128)],
                        rhs=qT_sb[:, h, ts(g, 512)],
                        start=True,
                        stop=True,
                    )
                    pt = pt_pool.tile([128, 512], BF16, name="pt")
                    nc.scalar.activation(pt[:, :], st[:, :], EXP, scale=INV_SQRT_D)
                    nc.tensor.matmul(
                        sums[:, :],
                        lhsT=ones_sb[:, :],
                        rhs=pt[:, :],
                        start=(c == 0),
                        stop=(c == KVCH - 1),
                    )
                    nc.tensor.matmul(
                        att[:, :],
                        lhsT=v_sb[:, c, :],
                        rhs=pt[:, :],
                        start=(c == 0),
                        stop=(c == KVCH - 1),
                    )
                recip = rc_pool.tile([128, 512], F32, name="recip")
                nc.vector.reciprocal_approx_fast(recip[:, :], sums[:, :])
                nc.vector.tensor_mul(
                    attnT_sb[:, h, ts(g, 512)], att[:, :], recip[:, :]
                )

        def stage_and_a2a(hs, nh, a_in, a_out):
            # stage attnT heads [hs, hs+nh) (both q halves) as 8 dest blocks,
            # A2A them, then unpack on gpsimd (queue: cc1,recv1,cc2,... —
            # a recv's wait on cc-completion only delays the next trigger by
            # its issue time, and the cc stream serializes anyway)
            for j in range(NCORES):
                nc.sync.dma_start(
                    out=a_in[ds(j * D, D), :],
                    in_=attnT_sb[:, ds(hs, nh), ds(j * OWN, OWN)],
                )
            nc.gpsimd.collective_compute(
                "AllToAll",
                mybir.AluOpType.bypass,
                ins=[a_in[:, :].opt()],
                outs=[a_out[:, :].opt()],
                replica_groups=[list(range(NCORES))],
            )
            for j in range(NCORES):
                nc.gpsimd.dma_start(
                    out=aT_sb[:, j, ds(hs, nh), :], in_=a_out[ds(j * D, D), :]
                )

        with tc.tile_pool(name="st_ps", bufs=4, space="PSUM") as st_ps, tc.tile_pool(
            name="sums_ps", bufs=2, space="PSUM"
        ) as sums_ps, tc.tile_pool(name="at_ps", bufs=2, space="PSUM") as at_ps:
            attention_head(0)
            attention_head(1)
            stage_and_a2a(0, 2, a2a1_in, a2a1_out)
            attention_head(2)
            stage_and_a2a(2, 1, a2a2_in, a2a2_out)
            attention_head(3)
            stage_and_a2a(3, 1, a2a3_in, a2a3_out)

        # ---- o_proj: own 128 rows, full 4096 contraction, 8 PSUM banks
        with tc.tile_pool(name="ob_ps", bufs=1, space="PSUM") as ob_ps:
            o_ps = [
                ob_ps.tile([128, 512], F32, tag=f"ob{n}", name=f"ob{n}")
                for n in range(H // 512)
            ]
            for k, (j, h) in enumerate(CONS):
                if k % 4 == 0 and k // 4 >= 4:
                    issue_wo_group(k // 4)
                wo_t = wo_tiles[k // 4]
                for n in range(H // 512):
                    nc.tensor.matmul(
                        o_ps[n][:, :],
                        lhsT=aT_sb[:, j, h, :],
                        rhs=wo_t[:, k % 4, ts(n, 512)],
                        start=(k == 0),
                        stop=(k == len(CONS) - 1),
                    )
            for n in range(H // 512):
                ob = out_pool.tile([128, 512], F32, tag="ob", name="ob")
                nc.vector.tensor_copy(ob[:, :], o_ps[n][:, :])
                nc.sync.dma_start(out=out_ext[:, ts(n, 512)], in_=ob[:, :])

    nc.finalize()
    return nc


def _get_nc():
    if "nc" not in _NC_CACHE:
        _NC_CACHE["nc"] = _build_nc()
    return _NC_CACHE["nc"]


def _rope_tables():
    inv_freq = 1.0 / (ROPE_THETA ** (np.arange(0, D, 2, dtype=np.float32) / D))
    pos = np.arange(KV, dtype=np.float32)
    freqs = pos[:, None] * inv_freq[None, :]  # [KV, D/2]
    emb = np.concatenate([freqs, freqs], axis=-1)  # [KV, D]
    return np.cos(emb), np.sin(emb)  # [KV, D]


def _host_rope(x, cos, sin):
    # x: [S, D]; cos/sin: [S, D]
    x1, x2 = x[:, : D // 2], x[:, D // 2 :]
    rot = np.concatenate([-x2, x1], axis=-1)
    return x * cos + rot * sin


def _pack_chunks(a):
    """[N*128, F] -> [128, N, F] with [p, c, f] = a[128c+p, f]."""
    n = a.shape[0] // 128
    return np.ascontiguousarray(a.reshape(n, 128, -1).transpose(1, 0, 2))


def kernel(hidden_states, past_k, past_v, Wq, Wk, Wv, Wo, trace=False):
    global LAST_RESULT
    bf = ml_dtypes.bfloat16
    x = np.asarray(hidden_states, dtype=np.float32)[0]  # [SQ, H]
    xT_p = _pack_chunks(np.ascontiguousarray(x.T)).astype(bf)  # [128, 32, 1024]
    cos, sin = _rope_tables()  # [KV, D] f32
    cosq = np.ascontiguousarray(cos[SP:].T).astype(bf)  # [128, 1024]
    sinq = np.ascontiguousarray(sin[SP:].T).astype(bf)
    woT_p = _pack_chunks(np.ascontiguousarray(np.asarray(Wo, dtype=np.float32).T)).astype(
        bf
    )  # [128, 32, 4096], full Wo, replicated

    Wq_n = np.asarray(Wq, dtype=np.float32)
    Wk_n = np.asarray(Wk, dtype=np.float32)
    Wv_n = np.asarray(Wv, dtype=np.float32)
    pk_n = np.asarray(past_k, dtype=np.float32)
    pv_n = np.asarray(past_v, dtype=np.float32)

    in_maps = []
    for m in range(NCORES):
        kr = slice(m * D, (m + 1) * D)
        wq_j = np.stack(
            [
                _pack_chunks(np.ascontiguousarray(Wq_n[m * DQ + j * D : m * DQ + (j + 1) * D].T))
                for j in range(HPC)
            ],
            axis=1,
        )  # [128, HPC, HCH, D]
        in_maps.append(
            {
                "xT": xT_p,
                "wkT": _pack_chunks(np.ascontiguousarray(Wk_n[kr].T)).astype(bf),
                "wvT": _pack_chunks(np.ascontiguousarray(Wv_n[kr].T)).astype(bf),
                "wqT": np.ascontiguousarray(wq_j).astype(bf),
                "woT": woT_p,
                "pkT": np.ascontiguousarray(
                    _host_rope(pk_n[0, m], cos[:SP], sin[:SP]).T
                ).astype(bf),
                "pv": _pack_chunks(np.ascontiguousarray(pv_n[0, m])).astype(bf),
                "cosq": cosq,
                "sinq": sinq,
            }
        )

    nc = _get_nc()
    res = run_bass_kernel_spmd(
        nc, in_maps, core_ids=list(range(NCORES)), trace=trace
    )
    LAST_RESULT = res
    out = np.empty((SQ, H), dtype=np.float32)
    for m in range(NCORES):
        out[m * OWN : (m + 1) * OWN] = np.asarray(
            res.results[m]["out"], dtype=np.float32
        )
    return out.reshape(B, SQ, H)
